# revision 21
# baseline (speedup 1.0000x reference)
"""Trainium2 Bass kernel for nn_CameraParamLoss.

Strategy:
  - Pure data parallel: N=131072 samples split across 8 cores (16384 each).
  - The per-sample 3x3 normal-equation solve has arrowhead structure
        [[a, b, c], [b, d, 0], [c, 0, d]]  with
        a = sum v*(x^2+y^2) + eps', b = sum v*x, c = sum v*y, d = sum v + eps
    so it is solved in closed form from 7 per-joint reductions.
  - The /1000 scaling of joints folds out exactly: working with raw joints,
    the eps on the (0,0) entry becomes eps/1e-6 = 100.0, and the final
    scale parameter p gets multiplied by 1000.
  - rel_trans_valid gating is handled by split sums over joints [0,21) and
    [21,42):  S = S_a + R * S_b  (exact for 0/1 masks), so no gating pass
    over the joint tensors is needed.
  - Layout: sample-major tiles [128 partitions, D samples, ...]; work is
    split across VectorE (products + reduces + solve), GpSimd (mask
    products, pair sums, loss), ScalarE/ACT (Square + output DMA issue).
  - Sync is left entirely to Tile + Bacc: waits above the 1-per-instruction
    ISA budget are split by Bacc onto EventSemaphore instructions
    (generate_event_semaphores). Hand-pinned engine-order chains are
    disabled by default — they deadlocked on hardware.
"""

import os
import sys

sys.path.insert(0, "/opt/trn_rl_repo")

import numpy as np

import concourse.bacc as bacc
import concourse.tile as tile
from concourse import mybir
from concourse.bass_utils import run_bass_kernel_spmd
from concourse.tile_rust import add_dep_helper

N_CORES = 8
N_TOTAL = 131072
NS_FULL = N_TOTAL // N_CORES  # 16384 samples per core
_NG_OVERRIDE = int(os.environ.get("KERNEL_NG", "0"))  # 0 = full
NS = NS_FULL if not _NG_OVERRIDE else _NG_OVERRIDE * 4 * 2048
NJ = 42
P = 128                  # SBUF partitions
D = 16                   # samples per partition per supertile
ST = P * D               # 2048 samples per supertile
NST = NS // ST           # supertiles per core
GS = 4                   # supertiles per solve/loss group
NG = NST // GS           # groups
F32 = mybir.dt.float32
ALU = mybir.AluOpType
ACTF = mybir.ActivationFunctionType
AX = mybir.AxisListType

# Bisection flags: route work away from suspect engines.
USE_GP = os.environ.get("KERNEL_NO_GP", "0") != "1"
USE_ACT = os.environ.get("KERNEL_NO_ACT", "0") != "1"
OUT_ON_ACT = os.environ.get("KERNEL_NO_OUTDMA_ACT", "0") != "1"
USE_NOPS = os.environ.get("KERNEL_NO_NOPS", "1") != "1"
USE_CHAINS = os.environ.get("KERNEL_NO_CHAINS", "1") != "1"

_PROGRAM = None


class _Chains:
    """Pin per-engine program order with no-sync dependency edges."""

    def __init__(self):
        self.last = {}

    def __call__(self, key, binst):
        if not USE_CHAINS:
            return binst
        prev = self.last.get(key)
        if prev is not None and binst is not None:
            add_dep_helper(
                binst.ins, prev.ins, sync=False, reason="engine order"
            )
        if binst is not None:
            self.last[key] = binst
        return binst


def _build_program():
    nc = bacc.Bacc(
        "TRN2",
        target_bir_lowering=False,
        debug=False,
        num_devices=N_CORES,
    )

    joints = nc.dram_tensor("joints", [NS, NJ, 3], F32, kind="ExternalInput")
    valid = nc.dram_tensor("valid", [NS, NJ], F32, kind="ExternalInput")
    loc2d = nc.dram_tensor("loc2d", [NS, NJ, 2], F32, kind="ExternalInput")
    pred = nc.dram_tensor("pred", [6, NS, 3], F32, kind="ExternalInput")
    rel = nc.dram_tensor("rel", [NS, 1], F32, kind="ExternalInput")
    out_ls = nc.dram_tensor("out_ls", [6, NS, 1], F32, kind="ExternalOutput")
    out_lt = nc.dram_tensor("out_lt", [6, NS, 2], F32, kind="ExternalOutput")
    out_cam = nc.dram_tensor("out_cam", [1, NS, 3], F32, kind="ExternalOutput")

    jap = joints.ap()
    vap = valid.ap()
    lap = loc2d.ap()
    pap = pred.ap()
    rap = rel.ap()
    lsap = out_ls.ap()
    ltap = out_lt.ap()
    camap = out_cam.ap()[0]  # [NS, 3]

    ve = nc.vector
    act = nc.scalar
    ch = _Chains()

    gp = nc.gpsimd if USE_GP else nc.vector
    gpk = "gp" if USE_GP else "ve"
    sqeng, sqk = (act, "act") if USE_ACT else (ve, "ve")
    oeng, ok_ = (act, "act") if OUT_ON_ACT else (nc.sync, "sp")

    def absorb(key, engine, dep_binst):
        """NOP that carries one cross-engine wait so the next DMA on this
        engine's stream only needs its DMA-lane wait."""
        if not USE_NOPS or not USE_CHAINS:
            return None
        nop = engine.nop(nofuse=True, hint="absorb")
        add_dep_helper(nop.ins, dep_binst.ins, reason="absorb wait")
        return ch(key, nop)

    t2a_hist = []
    t2b_hist = []

    # stats slots (width 16):
    #  A-half (j<21):  0:2=(Sx,Sy) 2:4=(f,g) 4=Sv 5=e 6=Sxx
    #  B-half (j>=21): 7:9         9:11      11   12  13
    #  14 = d scratch
    with tile.TileContext(nc) as tc:
        with (
            tc.tile_pool(name="inp", bufs=2) as inp,
            tc.tile_pool(name="work", bufs=2) as work,
            tc.tile_pool(name="grp", bufs=2) as grp,
            tc.tile_pool(name="slv", bufs=2) as slv,
        ):
            for g in range(NG):
                g0 = g * GS * ST
                stats = grp.tile([P, GS, D, 16], F32, tag="stats")
                ppred = grp.tile([P, GS, 6, D, 3], F32, tag="ppred")
                dtile = grp.tile([P, GS, 6, D, 3], F32, tag="dtile")
                cam = grp.tile([P, GS, D, 3], F32, tag="cam")
                cv = grp.tile([P, GS, D], F32, tag="cv")
                ss = grp.tile([P, GS, 6, D], F32, tag="ss")
                stg = grp.tile([P, GS, 6, D, 2], F32, tag="stg")
                rg = grp.tile([P, GS, D], F32, tag="rg")
                dumP = grp.tile([P, 1], F32, tag="dumP")
                dumV = grp.tile([P, 1], F32, tag="dumV")

                ch("sp", nc.sync.dma_start(
                    out=rg[:],
                    in_=rap[g0 : g0 + GS * ST].rearrange(
                        "(s p d) o -> p s (d o)", s=GS, p=P
                    ),
                ))

                for s in range(GS):
                    n0 = g0 + s * ST
                    k = g * GS + s  # global supertile index

                    J = inp.tile([P, D, NJ, 3], F32, tag="J")
                    V = inp.tile([P, D, NJ], F32, tag="V")
                    L = inp.tile([P, D, NJ, 2], F32, tag="L")

                    if k >= 2:
                        # absorb WAR hazards of this supertile's input slots
                        absorb("sp", nc.sync, t2a_hist[k - 2])
                        absorb("sp", nc.sync, t2b_hist[k - 2])
                    ch("sp", nc.sync.dma_start(
                        out=J[:],
                        in_=jap[n0 : n0 + ST].rearrange(
                            "(p d) j c -> p d j c", p=P
                        ),
                    ))
                    ch("sp", nc.sync.dma_start(
                        out=V[:],
                        in_=vap[n0 : n0 + ST].rearrange(
                            "(p d) j -> p d j", p=P
                        ),
                    ))
                    ch("sp", nc.sync.dma_start(
                        out=L[:],
                        in_=lap[n0 : n0 + ST].rearrange(
                            "(p d) j c -> p d j c", p=P
                        ),
                    ))
                    ch("sp", nc.sync.dma_start(
                        out=ppred[:, s],
                        in_=pap[:, n0 : n0 + ST, :].rearrange(
                            "k (p d) c -> p k d c", p=P
                        ),
                    ))

                    # ---- DVE stream ----
                    ch("ve", ve.reduce_sum(
                        stats[:, s, :, 4], V[:, :, 0:21], axis=AX.X
                    ))
                    ch("ve", ve.reduce_sum(
                        stats[:, s, :, 11], V[:, :, 21:NJ], axis=AX.X
                    ))
                    # vm = v * (x, y)
                    vm = work.tile([P, D, NJ, 2], F32, tag="vm")
                    ch("ve", ve.tensor_mul(
                        vm[:],
                        J[:, :, :, 0:2],
                        V[:].unsqueeze(3).broadcast_to([P, D, NJ, 2]),
                    ))
                    # (Sx, Sy) halves
                    ch("ve", ve.reduce_sum(
                        stats[:, s, :, 0:2],
                        vm[:, :, 0:21, :].transpose([0, 1, 3, 2]),
                        axis=AX.X,
                    ))
                    ch("ve", ve.reduce_sum(
                        stats[:, s, :, 7:9],
                        vm[:, :, 21:NJ, :].transpose([0, 1, 3, 2]),
                        axis=AX.X,
                    ))
                    # t2 = vm * (u, w) — first half on DVE
                    t2a = work.tile([P, D, 21, 2], F32, tag="t2a")
                    t2a_hist.append(ch("ve", ve.tensor_mul(
                        t2a[:], vm[:, :, 0:21, :], L[:, :, 0:21, :]
                    )))

                    # ---- ACT: sq = vm^2 ----
                    sq = work.tile([P, D, NJ, 2], F32, tag="sq")
                    if USE_ACT:
                        ch(sqk, sqeng.activation(sq[:], vm[:], ACTF.Square))
                    else:
                        ch("ve", ve.tensor_mul(sq[:], vm[:], vm[:]))

                    # ---- GP stream ----
                    if USE_GP:
                        ch(gpk, gp.tensor_copy(dumP[:], ppred[:, s, 0, 0, 0:1]))
                        ch(gpk, gp.tensor_copy(dumV[:], V[:, 0, 0:1]))
                    vmuw = work.tile([P, D, NJ, 2], F32, tag="vmuw")
                    ch(gpk, gp.tensor_mul(
                        vmuw[:],
                        L[:],
                        V[:].unsqueeze(3).broadcast_to([P, D, NJ, 2]),
                    ))
                    t2b = work.tile([P, D, 21, 2], F32, tag="t2b")
                    t2b_hist.append(ch(gpk, gp.tensor_mul(
                        t2b[:], vm[:, :, 21:NJ, :], L[:, :, 21:NJ, :]
                    )))
                    peb = work.tile([P, D, 21], F32, tag="peb")
                    ch(gpk, gp.tensor_add(
                        peb[:], t2b[:, :, :, 0], t2b[:, :, :, 1]
                    ))
                    pea = work.tile([P, D, 21], F32, tag="pea")
                    ch(gpk, gp.tensor_add(
                        pea[:], t2a[:, :, :, 0], t2a[:, :, :, 1]
                    ))

                    # ---- DVE reductions fed by GP/ACT ----
                    ch("ve", ve.reduce_sum(
                        stats[:, s, :, 2:4],
                        vmuw[:, :, 0:21, :].transpose([0, 1, 3, 2]),
                        axis=AX.X,
                    ))
                    ch("ve", ve.reduce_sum(
                        stats[:, s, :, 9:11],
                        vmuw[:, :, 21:NJ, :].transpose([0, 1, 3, 2]),
                        axis=AX.X,
                    ))
                    ch("ve", ve.reduce_sum(
                        stats[:, s, :, 6], sq[:, :, 0:21, :], axis=AX.XY
                    ))
                    ch("ve", ve.reduce_sum(
                        stats[:, s, :, 13], sq[:, :, 21:NJ, :], axis=AX.XY
                    ))
                    ch("ve", ve.reduce_sum(
                        stats[:, s, :, 12], peb[:], axis=AX.X
                    ))
                    ch("ve", ve.reduce_sum(
                        stats[:, s, :, 5], pea[:], axis=AX.X
                    ))

                # ---- combine split sums: S = S_a + R * S_b ----
                cmb = slv.tile([P, GS, D, 7], F32, tag="cmb")
                ch("ve", ve.tensor_mul(
                    cmb[:],
                    stats[:, :, :, 7:14],
                    rg[:].unsqueeze(3).broadcast_to([P, GS, D, 7]),
                ))
                ch("ve", ve.tensor_add(
                    stats[:, :, :, 0:7], stats[:, :, :, 0:7], cmb[:]
                ))

                sxy = stats[:, :, :, 0:2]
                sv = stats[:, :, :, 4]
                sd = stats[:, :, :, 14]

                # cam_valid first so later GP waits cover it
                ch("ve", ve.tensor_single_scalar(cv[:], sv, 0.0, op=ALU.is_gt))
                # d = Sv + eps ; a = Sxx + eps/s^2 (in place)
                ch("ve", ve.tensor_scalar_add(sd, sv, 1e-4))
                ch("ve", ve.tensor_scalar_add(
                    stats[:, :, :, 6], stats[:, :, :, 6], 100.0
                ))

                ad_ed = slv.tile([P, GS, D, 2], F32, tag="ad_ed")
                # (e*d, a*d)
                ch("ve", ve.tensor_mul(
                    ad_ed[:],
                    stats[:, :, :, 5:7],
                    sd.unsqueeze(3).broadcast_to([P, GS, D, 2]),
                ))
                # ((Sx^2, Sy^2), (Sx*f, Sy*g))
                sqbf = slv.tile([P, GS, D, 2, 2], F32, tag="sqbf")
                ch("ve", ve.tensor_mul(
                    sqbf[:],
                    sxy.unsqueeze(3).broadcast_to([P, GS, D, 2, 2]),
                    stats[:, :, :, 0:4].rearrange("p g d (r c) -> p g d r c", r=2),
                ))
                # (b2c2, bfcg)
                bc = slv.tile([P, GS, D, 2], F32, tag="bc")
                ch("ve", ve.tensor_add(
                    bc[:], sqbf[:, :, :, :, 0], sqbf[:, :, :, :, 1]
                ))

                # detnp[...,0] = np = e*d - bfcg ; [...,1] = det = a*d - b2c2
                detnp = slv.tile([P, GS, D, 2], F32, tag="detnp")
                ch("ve", ve.tensor_sub(
                    detnp[:, :, :, 0], ad_ed[:, :, :, 0], bc[:, :, :, 1]
                ))
                ch("ve", ve.tensor_sub(
                    detnp[:, :, :, 1], ad_ed[:, :, :, 1], bc[:, :, :, 0]
                ))

                dd = slv.tile([P, GS, D], F32, tag="dd")
                ch("ve", ve.tensor_mul(dd[:], detnp[:, :, :, 1], sd))
                inv = slv.tile([P, GS, D], F32, tag="inv")
                ch("ve", ve.reciprocal(inv[:], dd[:]))

                # p = 1000 * np / det = np * (d*1000) * inv
                d1000 = slv.tile([P, GS, D], F32, tag="d1000")
                ch("ve", ve.tensor_scalar(
                    d1000[:], sv, 1e-4, 1000.0, ALU.add, ALU.mult
                ))
                p1 = slv.tile([P, GS, D], F32, tag="p1")
                ch("ve", ve.tensor_mul(p1[:], detnp[:, :, :, 0], d1000[:]))
                ch("ve", ve.tensor_mul(cam[:, :, :, 0], p1[:], inv[:]))

                # (q, r) = ((f,g)*det - (Sx,Sy)*np) * inv
                tt4 = slv.tile([P, GS, D, 2, 2], F32, tag="tt4")
                ch("ve", ve.tensor_mul(
                    tt4[:],
                    stats[:, :, :, 0:4].rearrange("p g d (r c) -> p g d r c", r=2),
                    detnp[:].unsqueeze(4).broadcast_to([P, GS, D, 2, 2]),
                ))
                t3 = slv.tile([P, GS, D, 2], F32, tag="t3")
                ch("ve", ve.tensor_sub(
                    t3[:], tt4[:, :, :, 1, :], tt4[:, :, :, 0, :]
                ))
                qr_inst = ch("ve", ve.tensor_mul(
                    cam[:, :, :, 1:3],
                    t3[:],
                    inv[:].unsqueeze(3).broadcast_to([P, GS, D, 2]),
                ))

                # ---- loss: |pred - cam| * cv ----
                ch(gpk, gp.tensor_sub(
                    dtile[:],
                    ppred[:],
                    cam[:].unsqueeze(2).broadcast_to([P, GS, 6, D, 3]),
                ))
                if USE_ACT:
                    ch("act", act.activation(dtile[:], dtile[:], ACTF.Abs))
                else:
                    ch("ve", ve.tensor_single_scalar(
                        dtile[:], dtile[:], 0.0, op=ALU.abs_max
                    ))
                last_stt = None
                for s in range(GS):
                    ch(gpk, gp.tensor_mul(
                        ss[:, s],
                        dtile[:, s, :, :, 0],
                        cv[:, s].unsqueeze(1).broadcast_to([P, 6, D]),
                    ))
                    last_stt = ch(gpk, gp.tensor_mul(
                        stg[:, s],
                        dtile[:, s, :, :, 1:3],
                        cv[:, s].unsqueeze(1).unsqueeze(3).broadcast_to(
                            [P, 6, D, 2]
                        ),
                    ))

                # ---- outputs ----
                absorb(ok_, oeng, qr_inst)
                absorb(ok_, oeng, last_stt)
                for s in range(GS):
                    n0 = g0 + s * ST
                    ch(ok_, oeng.dma_start(
                        out=camap[n0 : n0 + ST].rearrange(
                            "(p d) c -> p d c", p=P
                        ),
                        in_=cam[:, s],
                    ))
                for s in range(GS):
                    n0 = g0 + s * ST
                    ch(ok_, oeng.dma_start(
                        out=lsap[:, n0 : n0 + ST, :].rearrange(
                            "k (p d) o -> p k (d o)", p=P
                        ),
                        in_=ss[:, s],
                    ))
                for s in range(GS):
                    n0 = g0 + s * ST
                    ch(ok_, oeng.dma_start(
                        out=ltap[:, n0 : n0 + ST, :].rearrange(
                            "k (p d) c -> p k d c", p=P
                        ),
                        in_=stg[:, s],
                    ))

    nc.finalize()
    return nc


def _get_program():
    global _PROGRAM
    if _PROGRAM is None:
        _PROGRAM = _build_program()
    return _PROGRAM


def kernel(**inputs):
    joints_gt = np.asarray(inputs["joints_gt"], dtype=np.float32)
    joint_valid_in = np.asarray(inputs["joint_valid_in"], dtype=np.float32)
    joint_loc_2d_gt = np.asarray(inputs["joint_loc_2d_gt"], dtype=np.float32)
    cam_param_pred = np.asarray(inputs["cam_param_pred"], dtype=np.float32)
    rel_trans_valid = np.asarray(inputs["rel_trans_valid"], dtype=np.float32)

    nc = _get_program()

    in_maps = []
    for i in range(N_CORES):
        lo, hi = i * NS, (i + 1) * NS
        in_maps.append(
            {
                "joints": np.ascontiguousarray(joints_gt[lo:hi]),
                "valid": np.ascontiguousarray(joint_valid_in[lo:hi]),
                "loc2d": np.ascontiguousarray(joint_loc_2d_gt[lo:hi]),
                "pred": np.ascontiguousarray(cam_param_pred[:, lo:hi, :]),
                "rel": np.ascontiguousarray(rel_trans_valid[lo:hi]),
            }
        )

    res = run_bass_kernel_spmd(nc, in_maps, core_ids=list(range(N_CORES)))

    loss_scale = np.concatenate([r["out_ls"] for r in res.results], axis=1)
    loss_trans = np.concatenate([r["out_lt"] for r in res.results], axis=1)
    cam_gt = np.concatenate([r["out_cam"] for r in res.results], axis=1)
    return (loss_scale, loss_trans, cam_gt)


# revision 23
# speedup vs baseline: 1.3812x; 1.3812x over previous
"""Trainium2 Bass kernel for nn_CameraParamLoss.

Strategy:
  - Pure data parallel: N=131072 samples split across 8 cores (16384 each).
  - The per-sample 3x3 normal-equation solve has arrowhead structure
        [[a, b, c], [b, d, 0], [c, 0, d]]  with
        a = sum v*(x^2+y^2) + eps', b = sum v*x, c = sum v*y, d = sum v + eps
    so it is solved in closed form from 7 per-joint reductions.
  - The /1000 scaling of joints folds out exactly: working with raw joints,
    the eps on the (0,0) entry becomes eps/1e-6 = 100.0, and the final
    scale parameter p gets multiplied by 1000.
  - rel_trans_valid gating is handled by split sums over joints [0,21) and
    [21,42):  S = S_a + R * S_b  (exact for 0/1 masks), so no gating pass
    over the joint tensors is needed.
  - Layout: sample-major tiles [128 partitions, D samples, ...]; work is
    split across VectorE (products + reduces + solve), GpSimd (mask
    products, pair sums, loss), ScalarE/ACT (Square + output DMA issue).
  - Sync is left entirely to Tile + Bacc: waits above the 1-per-instruction
    ISA budget are split by Bacc onto EventSemaphore instructions
    (generate_event_semaphores). Hand-pinned engine-order chains are
    disabled by default — they deadlocked on hardware.
"""

import os
import sys

sys.path.insert(0, "/opt/trn_rl_repo")

import numpy as np

import concourse.bacc as bacc
import concourse.tile as tile
from concourse import mybir
from concourse.bass_utils import run_bass_kernel_spmd
from concourse.tile_rust import add_dep_helper

N_CORES = 8
N_TOTAL = 131072
NS_FULL = N_TOTAL // N_CORES  # 16384 samples per core
_NG_OVERRIDE = int(os.environ.get("KERNEL_NG", "0"))  # 0 = full
NS = NS_FULL if not _NG_OVERRIDE else _NG_OVERRIDE * 4 * 2048
NJ = 42
P = 128                  # SBUF partitions
D = 16                   # samples per partition per supertile
ST = P * D               # 2048 samples per supertile
NST = NS // ST           # supertiles per core
GS = 4                   # supertiles per solve/loss group
NG = NST // GS           # groups
F32 = mybir.dt.float32
ALU = mybir.AluOpType
ACTF = mybir.ActivationFunctionType
AX = mybir.AxisListType

# Bisection flags: route work away from suspect engines.
USE_GP = os.environ.get("KERNEL_NO_GP", "0") != "1"
USE_ACT = os.environ.get("KERNEL_NO_ACT", "0") != "1"
OUT_ON_ACT = os.environ.get("KERNEL_NO_OUTDMA_ACT", "0") != "1"
USE_NOPS = os.environ.get("KERNEL_NO_NOPS", "1") != "1"
USE_CHAINS = os.environ.get("KERNEL_NO_CHAINS", "1") != "1"

_PROGRAM = None


class _Chains:
    """Pin per-engine program order with no-sync dependency edges."""

    def __init__(self):
        self.last = {}

    def __call__(self, key, binst):
        if not USE_CHAINS:
            return binst
        prev = self.last.get(key)
        if prev is not None and binst is not None:
            add_dep_helper(
                binst.ins, prev.ins, sync=False, reason="engine order"
            )
        if binst is not None:
            self.last[key] = binst
        return binst


def _build_program():
    nc = bacc.Bacc(
        "TRN2",
        target_bir_lowering=False,
        debug=False,
        num_devices=N_CORES,
    )

    joints = nc.dram_tensor("joints", [NS, NJ, 3], F32, kind="ExternalInput")
    valid = nc.dram_tensor("valid", [NS, NJ], F32, kind="ExternalInput")
    loc2d = nc.dram_tensor("loc2d", [NS, NJ, 2], F32, kind="ExternalInput")
    pred = nc.dram_tensor("pred", [6, NS, 3], F32, kind="ExternalInput")
    rel = nc.dram_tensor("rel", [NS, 1], F32, kind="ExternalInput")
    out_ls = nc.dram_tensor("out_ls", [6, NS, 1], F32, kind="ExternalOutput")
    out_lt = nc.dram_tensor("out_lt", [6, NS, 2], F32, kind="ExternalOutput")
    out_cam = nc.dram_tensor("out_cam", [1, NS, 3], F32, kind="ExternalOutput")

    jap = joints.ap()
    vap = valid.ap()
    lap = loc2d.ap()
    pap = pred.ap()
    rap = rel.ap()
    lsap = out_ls.ap()
    ltap = out_lt.ap()
    camap = out_cam.ap()[0]  # [NS, 3]

    ve = nc.vector
    act = nc.scalar
    ch = _Chains()

    gp = nc.gpsimd if USE_GP else nc.vector
    gpk = "gp" if USE_GP else "ve"
    sqeng, sqk = (act, "act") if USE_ACT else (ve, "ve")
    oeng, ok_ = (act, "act") if OUT_ON_ACT else (nc.sync, "sp")

    def absorb(key, engine, dep_binst):
        """NOP that carries one cross-engine wait so the next DMA on this
        engine's stream only needs its DMA-lane wait."""
        if not USE_NOPS or not USE_CHAINS:
            return None
        nop = engine.nop(nofuse=True, hint="absorb")
        add_dep_helper(nop.ins, dep_binst.ins, reason="absorb wait")
        return ch(key, nop)

    t2a_hist = []
    t2b_hist = []

    # stats slots (width 16):
    #  A-half (j<21):  0:2=(Sx,Sy) 2:4=(f,g) 4=Sv 5=e 6=Sxx
    #  B-half (j>=21): 7:9         9:11      11   12  13
    #  14 = d scratch
    with tile.TileContext(nc) as tc:
        with (
            tc.tile_pool(name="inp", bufs=2) as inp,
            tc.tile_pool(name="work", bufs=2) as work,
            tc.tile_pool(name="grp", bufs=2) as grp,
            tc.tile_pool(name="slv", bufs=2) as slv,
        ):
            for g in range(NG):
                g0 = g * GS * ST
                stats = grp.tile([P, GS, D, 16], F32, tag="stats")
                ppred = grp.tile([P, GS, 6, D, 3], F32, tag="ppred")
                dtile = grp.tile([P, GS, 6, D, 3], F32, tag="dtile")
                cam = grp.tile([P, GS, D, 3], F32, tag="cam")
                cv = grp.tile([P, GS, D], F32, tag="cv")
                ss = grp.tile([P, GS, 6, D], F32, tag="ss")
                stg = grp.tile([P, GS, 6, D, 2], F32, tag="stg")
                rg = grp.tile([P, GS, D], F32, tag="rg")
                dumP = grp.tile([P, 1], F32, tag="dumP")
                dumV = grp.tile([P, 1], F32, tag="dumV")

                ch("sp", nc.sync.dma_start(
                    out=rg[:],
                    in_=rap[g0 : g0 + GS * ST].rearrange(
                        "(s p d) o -> p s (d o)", s=GS, p=P
                    ),
                ))

                for s in range(GS):
                    n0 = g0 + s * ST
                    k = g * GS + s  # global supertile index

                    J = inp.tile([P, D, NJ, 3], F32, tag="J")
                    V = inp.tile([P, D, NJ], F32, tag="V")
                    L = inp.tile([P, D, NJ, 2], F32, tag="L")

                    if k >= 2:
                        # absorb WAR hazards of this supertile's input slots
                        absorb("sp", nc.sync, t2a_hist[k - 2])
                        absorb("sp", nc.sync, t2b_hist[k - 2])
                    ch("sp", nc.sync.dma_start(
                        out=J[:],
                        in_=jap[n0 : n0 + ST].rearrange(
                            "(p d) j c -> p d j c", p=P
                        ),
                    ))
                    ch("sp", nc.sync.dma_start(
                        out=V[:],
                        in_=vap[n0 : n0 + ST].rearrange(
                            "(p d) j -> p d j", p=P
                        ),
                    ))
                    ch("sp", nc.sync.dma_start(
                        out=L[:],
                        in_=lap[n0 : n0 + ST].rearrange(
                            "(p d) j c -> p d j c", p=P
                        ),
                    ))
                    ch("sp", nc.sync.dma_start(
                        out=ppred[:, s],
                        in_=pap[:, n0 : n0 + ST, :].rearrange(
                            "k (p d) c -> p k d c", p=P
                        ),
                    ))

                    # ---- DVE stream ----
                    ch("ve", ve.reduce_sum(
                        stats[:, s, :, 4], V[:, :, 0:21], axis=AX.X
                    ))
                    ch("ve", ve.reduce_sum(
                        stats[:, s, :, 11], V[:, :, 21:NJ], axis=AX.X
                    ))
                    # vm = v * (x, y)
                    vm = work.tile([P, D, NJ, 2], F32, tag="vm")
                    ch("ve", ve.tensor_mul(
                        vm[:],
                        J[:, :, :, 0:2],
                        V[:].unsqueeze(3).broadcast_to([P, D, NJ, 2]),
                    ))
                    # (Sx, Sy) halves
                    ch("ve", ve.reduce_sum(
                        stats[:, s, :, 0:2],
                        vm[:, :, 0:21, :].transpose([0, 1, 3, 2]),
                        axis=AX.X,
                    ))
                    ch("ve", ve.reduce_sum(
                        stats[:, s, :, 7:9],
                        vm[:, :, 21:NJ, :].transpose([0, 1, 3, 2]),
                        axis=AX.X,
                    ))
                    # t2 = vm * (u, w) — first half on DVE
                    t2a = work.tile([P, D, 21, 2], F32, tag="t2a")
                    t2a_hist.append(ch("ve", ve.tensor_mul(
                        t2a[:], vm[:, :, 0:21, :], L[:, :, 0:21, :]
                    )))

                    # ---- ACT: sq = vm^2 ----
                    sq = work.tile([P, D, NJ, 2], F32, tag="sq")
                    if USE_ACT:
                        ch(sqk, sqeng.activation(sq[:], vm[:], ACTF.Square))
                    else:
                        ch("ve", ve.tensor_mul(sq[:], vm[:], vm[:]))

                    # ---- GP stream ----
                    if USE_GP:
                        ch(gpk, gp.tensor_copy(dumP[:], ppred[:, s, 0, 0, 0:1]))
                        ch(gpk, gp.tensor_copy(dumV[:], V[:, 0, 0:1]))
                    vmuw = work.tile([P, D, NJ, 2], F32, tag="vmuw")
                    ch(gpk, gp.tensor_mul(
                        vmuw[:],
                        L[:],
                        V[:].unsqueeze(3).broadcast_to([P, D, NJ, 2]),
                    ))
                    t2b = work.tile([P, D, 21, 2], F32, tag="t2b")
                    t2b_hist.append(ch(gpk, gp.tensor_mul(
                        t2b[:], vm[:, :, 21:NJ, :], L[:, :, 21:NJ, :]
                    )))
                    peb = work.tile([P, D, 21], F32, tag="peb")
                    ch(gpk, gp.tensor_add(
                        peb[:], t2b[:, :, :, 0], t2b[:, :, :, 1]
                    ))
                    pea = work.tile([P, D, 21], F32, tag="pea")
                    ch(gpk, gp.tensor_add(
                        pea[:], t2a[:, :, :, 0], t2a[:, :, :, 1]
                    ))

                    # ---- DVE reductions fed by GP/ACT ----
                    ch("ve", ve.reduce_sum(
                        stats[:, s, :, 2:4],
                        vmuw[:, :, 0:21, :].transpose([0, 1, 3, 2]),
                        axis=AX.X,
                    ))
                    ch("ve", ve.reduce_sum(
                        stats[:, s, :, 9:11],
                        vmuw[:, :, 21:NJ, :].transpose([0, 1, 3, 2]),
                        axis=AX.X,
                    ))
                    ch("ve", ve.reduce_sum(
                        stats[:, s, :, 6], sq[:, :, 0:21, :], axis=AX.XY
                    ))
                    ch("ve", ve.reduce_sum(
                        stats[:, s, :, 13], sq[:, :, 21:NJ, :], axis=AX.XY
                    ))
                    ch("ve", ve.reduce_sum(
                        stats[:, s, :, 12], peb[:], axis=AX.X
                    ))
                    ch("ve", ve.reduce_sum(
                        stats[:, s, :, 5], pea[:], axis=AX.X
                    ))

                # ---- combine split sums: S = S_a + R * S_b ----
                cmb = slv.tile([P, GS, D, 7], F32, tag="cmb")
                ch("ve", ve.tensor_mul(
                    cmb[:],
                    stats[:, :, :, 7:14],
                    rg[:].unsqueeze(3).broadcast_to([P, GS, D, 7]),
                ))
                ch("ve", ve.tensor_add(
                    stats[:, :, :, 0:7], stats[:, :, :, 0:7], cmb[:]
                ))

                sxy = stats[:, :, :, 0:2]
                sv = stats[:, :, :, 4]
                sd = stats[:, :, :, 14]

                # cam_valid first so later GP waits cover it
                ch("ve", ve.tensor_single_scalar(cv[:], sv, 0.0, op=ALU.is_gt))
                # d = Sv + eps ; a = Sxx + eps/s^2 (in place)
                ch("ve", ve.tensor_scalar_add(sd, sv, 1e-4))
                ch("ve", ve.tensor_scalar_add(
                    stats[:, :, :, 6], stats[:, :, :, 6], 100.0
                ))

                ad_ed = slv.tile([P, GS, D, 2], F32, tag="ad_ed")
                # (e*d, a*d)
                ch("ve", ve.tensor_mul(
                    ad_ed[:],
                    stats[:, :, :, 5:7],
                    sd.unsqueeze(3).broadcast_to([P, GS, D, 2]),
                ))
                # ((Sx^2, Sy^2), (Sx*f, Sy*g))
                sqbf = slv.tile([P, GS, D, 2, 2], F32, tag="sqbf")
                ch("ve", ve.tensor_mul(
                    sqbf[:],
                    sxy.unsqueeze(3).broadcast_to([P, GS, D, 2, 2]),
                    stats[:, :, :, 0:4].rearrange("p g d (r c) -> p g d r c", r=2),
                ))
                # (b2c2, bfcg)
                bc = slv.tile([P, GS, D, 2], F32, tag="bc")
                ch("ve", ve.tensor_add(
                    bc[:], sqbf[:, :, :, :, 0], sqbf[:, :, :, :, 1]
                ))

                # detnp[...,0] = np = e*d - bfcg ; [...,1] = det = a*d - b2c2
                detnp = slv.tile([P, GS, D, 2], F32, tag="detnp")
                ch("ve", ve.tensor_sub(
                    detnp[:, :, :, 0], ad_ed[:, :, :, 0], bc[:, :, :, 1]
                ))
                ch("ve", ve.tensor_sub(
                    detnp[:, :, :, 1], ad_ed[:, :, :, 1], bc[:, :, :, 0]
                ))

                dd = slv.tile([P, GS, D], F32, tag="dd")
                ch("ve", ve.tensor_mul(dd[:], detnp[:, :, :, 1], sd))
                inv = slv.tile([P, GS, D], F32, tag="inv")
                ch("ve", ve.reciprocal(inv[:], dd[:]))

                # p = 1000 * np / det = np * (d*1000) * inv
                d1000 = slv.tile([P, GS, D], F32, tag="d1000")
                ch("ve", ve.tensor_scalar(
                    d1000[:], sv, 1e-4, 1000.0, ALU.add, ALU.mult
                ))
                p1 = slv.tile([P, GS, D], F32, tag="p1")
                ch("ve", ve.tensor_mul(p1[:], detnp[:, :, :, 0], d1000[:]))
                ch("ve", ve.tensor_mul(cam[:, :, :, 0], p1[:], inv[:]))

                # (q, r) = ((f,g)*det - (Sx,Sy)*np) * inv
                tt4 = slv.tile([P, GS, D, 2, 2], F32, tag="tt4")
                ch("ve", ve.tensor_mul(
                    tt4[:],
                    stats[:, :, :, 0:4].rearrange("p g d (r c) -> p g d r c", r=2),
                    detnp[:].unsqueeze(4).broadcast_to([P, GS, D, 2, 2]),
                ))
                t3 = slv.tile([P, GS, D, 2], F32, tag="t3")
                ch("ve", ve.tensor_sub(
                    t3[:], tt4[:, :, :, 1, :], tt4[:, :, :, 0, :]
                ))
                qr_inst = ch("ve", ve.tensor_mul(
                    cam[:, :, :, 1:3],
                    t3[:],
                    inv[:].unsqueeze(3).broadcast_to([P, GS, D, 2]),
                ))

                # ---- loss: |pred - cam| * cv ----
                ch(gpk, gp.tensor_sub(
                    dtile[:],
                    ppred[:],
                    cam[:].unsqueeze(2).broadcast_to([P, GS, 6, D, 3]),
                ))
                if USE_ACT:
                    ch("act", act.activation(dtile[:], dtile[:], ACTF.Abs))
                else:
                    ch("ve", ve.tensor_single_scalar(
                        dtile[:], dtile[:], 0.0, op=ALU.abs_max
                    ))
                last_stt = None
                for s in range(GS):
                    ch(gpk, gp.tensor_mul(
                        ss[:, s],
                        dtile[:, s, :, :, 0],
                        cv[:, s].unsqueeze(1).broadcast_to([P, 6, D]),
                    ))
                    last_stt = ch(gpk, gp.tensor_mul(
                        stg[:, s],
                        dtile[:, s, :, :, 1:3],
                        cv[:, s].unsqueeze(1).unsqueeze(3).broadcast_to(
                            [P, 6, D, 2]
                        ),
                    ))

                # ---- outputs ----
                absorb(ok_, oeng, qr_inst)
                absorb(ok_, oeng, last_stt)
                for s in range(GS):
                    n0 = g0 + s * ST
                    ch(ok_, oeng.dma_start(
                        out=camap[n0 : n0 + ST].rearrange(
                            "(p d) c -> p d c", p=P
                        ),
                        in_=cam[:, s],
                    ))
                for s in range(GS):
                    n0 = g0 + s * ST
                    ch(ok_, oeng.dma_start(
                        out=lsap[:, n0 : n0 + ST, :].rearrange(
                            "k (p d) o -> p k (d o)", p=P
                        ),
                        in_=ss[:, s],
                    ))
                for s in range(GS):
                    n0 = g0 + s * ST
                    ch(ok_, oeng.dma_start(
                        out=ltap[:, n0 : n0 + ST, :].rearrange(
                            "k (p d) c -> p k d c", p=P
                        ),
                        in_=stg[:, s],
                    ))

    nc.finalize()
    return nc


def _get_program():
    global _PROGRAM
    if _PROGRAM is None:
        _PROGRAM = _build_program()
    return _PROGRAM


def kernel(**inputs):
    joints_gt = np.asarray(inputs["joints_gt"], dtype=np.float32)
    joint_valid_in = np.asarray(inputs["joint_valid_in"], dtype=np.float32)
    joint_loc_2d_gt = np.asarray(inputs["joint_loc_2d_gt"], dtype=np.float32)
    cam_param_pred = np.asarray(inputs["cam_param_pred"], dtype=np.float32)
    rel_trans_valid = np.asarray(inputs["rel_trans_valid"], dtype=np.float32)

    nc = _get_program()

    in_maps = []
    for i in range(N_CORES):
        lo, hi = i * NS, (i + 1) * NS
        in_maps.append(
            {
                "joints": np.ascontiguousarray(joints_gt[lo:hi]),
                "valid": np.ascontiguousarray(joint_valid_in[lo:hi]),
                "loc2d": np.ascontiguousarray(joint_loc_2d_gt[lo:hi]),
                "pred": np.ascontiguousarray(cam_param_pred[:, lo:hi, :]),
                "rel": np.ascontiguousarray(rel_trans_valid[lo:hi]),
            }
        )

    res = run_bass_kernel_spmd(nc, in_maps, core_ids=list(range(N_CORES)))

    loss_scale = np.concatenate([r["out_ls"] for r in res.results], axis=1)
    loss_trans = np.concatenate([r["out_lt"] for r in res.results], axis=1)
    cam_gt = np.concatenate([r["out_cam"] for r in res.results], axis=1)
    return (loss_scale, loss_trans, cam_gt)
